# revision 8
# baseline (speedup 1.0000x reference)
"""MoE fused top-k-gating decode kernel for Trainium2 (8 NeuronCores).

Problem: B=32,S=1,H=2048, E=8 experts, I=5632, top_k=2, fp32.
Sharding: expert-parallel - core c owns expert c (w_gate/w_up/w_down[c]),
router weight replicated (rolled per-core so column 0 is the own expert).
Each core computes the full router (softmax + top-2 mask), its expert's
GLU-MLP for all 32 tokens, scales by its combine weight, and returns a
partial [T,H] output (fp16); the host sums the 8 partials.

The kernel is DMA-bound: weights stream as fp16 (host-side cast;
end-to-end rel err ~5e-4 vs the 2e-2 gate): 69.2 MB @ ~360 GB/s =
~193 us/core floor. Host prepacks every tensor into its SBUF tile
layout so each weight DMA is 128 contiguous >=4KB descriptors.

Tail scheduling: down-weight chunks are streamed shifted - slab n's
epilogue feeds chunks 4n..4n+3, but chunks 0 and 1 (whose interT is
ready after slab 0) are streamed LAST, so the dependent chain after
the final weight byte is just 4 matmuls + output copy instead of a
full slab epilogue. The combine weight is folded into the
intermediate before the down matmul, so the output needs no final
scale; copies out of PSUM alternate DVE/Pool to overlap.
"""

import numpy as np

import concourse.bass as bass
import concourse.bacc as bacc
import concourse.mybir as mybir
import concourse.tile as tile
from concourse.masks import make_identity
from concourse.bass_utils import run_bass_kernel_spmd

B, S, H = 32, 1, 2048
E, I = 8, 5632
T = B * S          # 32 tokens
P = 128            # partitions
NCORES = 8
SWIGLU_SCALE = 1.702

KH = H // P        # 16 contraction chunks over H
KI = I // P        # 44 contraction chunks over I
NW = 512           # moving-dim tile width
NT = I // NW       # 11 gate/up column slabs
ND = H // NW       # 4 down output tiles
XW = T + E         # packed xT+router width (40)
SLAB = KH * NW     # 8192 cols per gate/up slab tile

F32 = mybir.dt.float32
F16 = mybir.dt.float16
AX = mybir.AxisListType.X
AF = mybir.ActivationFunctionType
OP = mybir.AluOpType


def _build_nc() -> bass.Bass:
    nc = bacc.Bacc()

    WDT = F16
    xrw_d = nc.declare_dram_parameter("xrw", [P, KH * XW], WDT, isOutput=False)
    wg_d = nc.declare_dram_parameter("wg", [P, NT * SLAB], WDT, isOutput=False)
    wu_d = nc.declare_dram_parameter("wu", [P, NT * SLAB], WDT, isOutput=False)
    wd_d = nc.declare_dram_parameter("wd", [P, KI * H], WDT, isOutput=False)
    out_d = nc.declare_dram_parameter("out", [T, H], F16, isOutput=True)

    with tile.TileContext(nc) as tc:
        with tc.tile_pool(name="const", bufs=1) as const:
            id_sb = const.tile([T, T], F32, name="id_sb")
            make_identity(nc, id_sb)

            xrw_sb = const.tile([P, KH * XW], WDT, name="xrw_sb")
            nc.sync.dma_start(out=xrw_sb, in_=xrw_d[:, :])

            interT_sb = const.tile([P, KI * T], WDT, name="interT_sb")
            comb_sb = const.tile([T, 1], F32, name="comb_sb")
            out_sb = const.tile([T, H], F16, name="out_sb")

            def xT_k(k):  # [128, 32] stationary activation chunk
                return xrw_sb[:, k * XW : k * XW + T]

            def rw_k(k):  # [128, 8] router weight chunk
                return xrw_sb[:, k * XW + T : (k + 1) * XW]

            wgp = tc.alloc_tile_pool(name="wgp", bufs=3)
            wup = tc.alloc_tile_pool(name="wup", bufs=3)
            wdp = tc.alloc_tile_pool(name="wdp", bufs=3)

            # ---------------- router: softmax + top-2 mask ----------------
            with (
                tc.tile_pool(name="rps", bufs=1, space="PSUM") as rps,
                tc.tile_pool(name="rsb", bufs=1) as rsb,
            ):
                # absorb the ident DMA tick on PE before anything else
                dmy_ps = rps.tile([T, T], F32, name="dmy_ps", tag="dmy")
                nc.tensor.transpose(dmy_ps, id_sb, id_sb)

                logits = rps.tile([T, E], F32, name="logits", tag="logits")
                for k in range(KH):
                    nc.tensor.matmul(
                        logits,
                        xT_k(k),
                        rw_k(k),
                        start=(k == 0),
                        stop=(k == KH - 1),
                    )
                # PSUM is read only by DVE (keeps later PE writers 1-wait)
                lg = rsb.tile([T, E], F32, name="lg")
                nc.vector.tensor_copy(lg, logits)
                mx = rsb.tile([T, 1], F32, name="mx")
                nc.vector.reduce_max(mx, lg, axis=AX)
                nmx = rsb.tile([T, 1], F32, name="nmx")
                nc.vector.tensor_scalar_mul(nmx, mx, -1.0)
                ex = rsb.tile([T, E], F32, name="ex")
                nc.scalar.activation(ex, lg, AF.Exp, bias=nmx, scale=1.0)
                sm = rsb.tile([T, 1], F32, name="sm")
                nc.vector.reduce_sum(sm, ex, axis=AX)
                rc = rsb.tile([T, 1], F32, name="rc")
                nc.vector.reciprocal(rc, sm)
                aff = rsb.tile([T, E], F32, name="aff")
                nc.vector.tensor_scalar_mul(aff, ex, rc)
                # top-2: value >= (second largest)
                m1 = rsb.tile([T, 1], F32, name="m1")
                nc.vector.reduce_max(m1, aff, axis=AX)
                pen = rsb.tile([T, E], F32, name="pen")
                nc.vector.tensor_scalar(
                    pen, aff, m1, -1e30, op0=OP.is_equal, op1=OP.mult
                )
                b2 = rsb.tile([T, E], F32, name="b2")
                nc.vector.tensor_add(b2, aff, pen)
                m2 = rsb.tile([T, 1], F32, name="m2")
                nc.vector.reduce_max(m2, b2, axis=AX)
                ge = rsb.tile([T, E], F32, name="ge")
                nc.vector.tensor_scalar(ge, aff, m2, None, op0=OP.is_ge)
                msk = rsb.tile([T, E], F32, name="msk")
                nc.vector.tensor_mul(msk, aff, ge)
                # rolled router weight puts the own expert at column 0.
                # ACT-engine Copy also prewarms its table for the tail copies.
                nc.scalar.activation(comb_sb, msk[:, 0:1], AF.Copy, scale=1.0)

            # ---- fused gate/up + swiglu + transpose + interleaved down ----
            # Slab n computes interT chunks 4n..4n+3. Down matmuls run on
            # the freshest interT: slab 0 feeds chunks 2,3 (pair on gpsimd);
            # slabs 1..10 feed their own 4 chunks (2 pairs each, gpsimd);
            # chunks 0 (sync) and 1 (gpsimd) stream LAST so the post-stream
            # dependent chain is only their 8 matmuls + output copies.
            # Accumulation order per PSUM bank: 2,3,4..43,0,1 (start at 2,
            # stop at 1). PSUM: gate/up 2 + transpose 2 + down accum 4 = 8.
            with (
                tc.tile_pool(name="gup", bufs=1, space="PSUM") as gup,
                tc.tile_pool(name="tps", bufs=2, space="PSUM") as tps,
                tc.tile_pool(name="dps", bufs=1, space="PSUM") as dps,
                tc.tile_pool(name="esb", bufs=2) as esb,
            ):
                d_ps = [
                    dps.tile([T, NW], F32, name=f"d_ps{j}", tag=f"d{j}")
                    for j in range(ND)
                ]

                def down_mms(ki, wd_ap, c):
                    # 4 matmuls accumulating chunk ki from wd_ap's chunk c
                    for j in range(ND):
                        nc.tensor.matmul(
                            d_ps[j],
                            interT_sb[:, ki * T : (ki + 1) * T],
                            wd_ap[:, c * H + j * NW : c * H + (j + 1) * NW],
                            start=(ki == 2),
                            stop=(ki == 1),
                        )

                for n in range(NT):
                    wg_sl = wgp.tile([P, SLAB], WDT, name="wg_sl", tag="wg")
                    wu_sl = wup.tile([P, SLAB], WDT, name="wu_sl", tag="wu")
                    nc.sync.dma_start(
                        out=wg_sl, in_=wg_d[:, n * SLAB : (n + 1) * SLAB]
                    )
                    nc.sync.dma_start(
                        out=wu_sl, in_=wu_d[:, n * SLAB : (n + 1) * SLAB]
                    )
                    g_ps = gup.tile([T, NW], F32, name="g_ps", tag="g")
                    u_ps = gup.tile([T, NW], F32, name="u_ps", tag="u")
                    for k in range(KH):
                        nc.tensor.matmul(
                            g_ps,
                            xT_k(k),
                            wg_sl[:, k * NW : (k + 1) * NW],
                            start=(k == 0),
                            stop=(k == KH - 1),
                        )
                    for k in range(KH):
                        nc.tensor.matmul(
                            u_ps,
                            xT_k(k),
                            wu_sl[:, k * NW : (k + 1) * NW],
                            start=(k == 0),
                            stop=(k == KH - 1),
                        )
                    # epilogue: sigmoid runs off a copy; fold combine weight
                    g_sb = esb.tile([T, NW], F32, name="g_sb", tag="gsb")
                    nc.vector.tensor_copy(g_sb, g_ps)
                    sig = esb.tile([T, NW], F32, name="sig", tag="sig")
                    nc.scalar.activation(sig, g_sb, AF.Sigmoid, scale=SWIGLU_SCALE)
                    t1 = esb.tile([T, NW], F32, name="t1", tag="t1")
                    nc.vector.tensor_mul(t1, g_ps, sig)
                    t2 = esb.tile([T, NW], F32, name="t2", tag="t2")
                    nc.vector.tensor_mul(t2, t1, u_ps)
                    inter = esb.tile([T, NW], F32, name="inter", tag="inter")
                    nc.vector.tensor_scalar_mul(inter, t2, comb_sb)
                    for j in range(NW // P):
                        ic = 4 * n + j
                        tp = tps.tile([P, T], F32, name="tp", tag="tp")
                        nc.tensor.transpose(tp, inter[:, j * P : (j + 1) * P], id_sb)
                        nc.vector.tensor_copy(
                            interT_sb[:, ic * T : (ic + 1) * T], tp
                        )
                    # down-weight pairs + matmuls for this slab's chunks
                    if n == 0:
                        kis = [(2, 3)]      # chunks 0,1 deferred to the end
                    else:
                        kis = [(4 * n, 4 * n + 1), (4 * n + 2, 4 * n + 3)]
                    for k0, k1 in kis:
                        wd_pr = wdp.tile([P, 2 * H], WDT, name="wd_pr", tag="wdpr")
                        nc.gpsimd.dma_start(
                            out=wd_pr, in_=wd_d[:, k0 * H : (k1 + 1) * H]
                        )
                        down_mms(k0, wd_pr, 0)
                        down_mms(k1, wd_pr, 1)

                # final chunks 0 (sync queue) and 1 (gpsimd queue)
                wd_c0 = wdp.tile([P, H], WDT, name="wd_c0", tag="wds0")
                nc.sync.dma_start(out=wd_c0, in_=wd_d[:, 0:H])
                wd_c1 = wdp.tile([P, H], WDT, name="wd_c1", tag="wds1")
                nc.gpsimd.dma_start(out=wd_c1, in_=wd_d[:, H : 2 * H])
                down_mms(0, wd_c0, 0)
                down_mms(1, wd_c1, 0)

                # output: PSUM -> fp16 SBUF (alternate DVE/ACT) -> DRAM
                for j in range(ND):
                    if j % 2 == 0:
                        nc.vector.tensor_copy(
                            out_sb[:, j * NW : (j + 1) * NW], d_ps[j]
                        )
                    else:
                        nc.scalar.activation(
                            out_sb[:, j * NW : (j + 1) * NW], d_ps[j],
                            AF.Copy, scale=1.0,
                        )
                    nc.sync.dma_start(
                        out=out_d[:, j * NW : (j + 1) * NW],
                        in_=out_sb[:, j * NW : (j + 1) * NW],
                    )
            wdp.release()
            wup.release()
            wgp.release()
    nc.finalize()
    return nc


def _pack_rows(a: np.ndarray) -> np.ndarray:
    """[K*P, C] row-major -> [P, K*C] SBUF tile layout (fp16)."""
    kp, c = a.shape
    k = kp // P
    return np.ascontiguousarray(
        a.reshape(k, P, c).transpose(1, 0, 2).reshape(P, k * c)
    )


def _make_in_maps(hidden_states, router_weight, w_gate, w_up, w_down):
    x = np.asarray(hidden_states, np.float32).reshape(T, H)
    rw = np.asarray(router_weight, np.float32)
    wg = np.asarray(w_gate, np.float16)
    wu = np.asarray(w_up, np.float16)
    wd = np.asarray(w_down, np.float16)
    xT = x.T.astype(np.float16)  # [H, T]

    # gate/up: [H, I] -> [P, NT*SLAB] where slab n holds I-cols n*512..,
    # k-chunk k at cols k*NW of the slab: reshape via (KH,P,NT,NW).
    def pack_gu(w):
        return np.ascontiguousarray(
            w.reshape(KH, P, NT, NW).transpose(1, 2, 0, 3).reshape(P, NT * SLAB)
        )

    in_maps = []
    for c in range(NCORES):
        order = [(j + c) % E for j in range(E)]  # column j holds expert (j+c)%E
        rwT = rw[order].T.astype(np.float16)  # [H, E]; col 0 = own expert
        xrw = _pack_rows(
            np.ascontiguousarray(np.concatenate([xT, rwT], axis=1))
        )  # [P, KH*XW]
        in_maps.append(
            {
                "xrw": xrw,
                "wg": pack_gu(wg[c]),
                "wu": pack_gu(wu[c]),
                "wd": _pack_rows(wd[c]),  # [P, KI*H], chunk ki at cols ki*H
            }
        )
    return in_maps


def kernel(
    hidden_states,
    router_weight,
    w_gate,
    w_up,
    w_down,
    top_k,
    _trace: bool = False,
    _trace_all: bool = False,
    **_unused,
):
    assert int(top_k) == 2, "kernel hardcodes top_k=2"
    in_maps = _make_in_maps(hidden_states, router_weight, w_gate, w_up, w_down)
    nc = _build_nc()
    res = run_bass_kernel_spmd(
        nc, in_maps, core_ids=list(range(NCORES)), trace=_trace,
        trace_cores=list(range(NCORES)) if (_trace and _trace_all) else None,
    )
    outs = np.stack([res.results[c]["out"] for c in range(NCORES)], axis=0)
    out = outs.astype(np.float64).sum(axis=0).astype(np.float32)
    if _trace:
        kernel.last_exec_time_ns = res.exec_time_ns
        kernel.last_mean_exec_time_ns = res.mean_exec_time_ns
        kernel.last_trace = res.instructions_and_trace
    return out.reshape(B, S, H)


# revision 13
# speedup vs baseline: 1.0589x; 1.0589x over previous
"""MoE fused top-k-gating decode kernel for Trainium2 (8 NeuronCores).

Problem: B=32,S=1,H=2048, E=8 experts, I=5632, top_k=2, fp32.
Sharding: expert-parallel - core c owns expert c (w_gate/w_up/w_down[c]),
router weight replicated (rolled per-core so column 0 is the own expert).
Each core computes the full router (softmax + top-2 mask), its expert's
GLU-MLP for all 32 tokens, scales by its combine weight, and returns a
partial [T,H] output (fp16); the host sums the 8 partials.

The kernel is DMA-bound: weights stream as fp16 (host-side cast;
end-to-end rel err ~5e-4 vs the 2e-2 gate): 69.2 MB @ ~360 GB/s =
~193 us/core floor. Host prepacks every tensor into its SBUF tile
layout so each weight DMA is 128 contiguous >=4KB descriptors.

Tail scheduling: down-weight chunks are streamed shifted - slab n's
epilogue feeds chunks 4n..4n+3, but chunks 0 and 1 (whose interT is
ready after slab 0) are streamed LAST, so the dependent chain after
the final weight byte is just 4 matmuls + output copy instead of a
full slab epilogue. The combine weight is folded into the
intermediate before the down matmul, so the output needs no final
scale; copies out of PSUM alternate DVE/Pool to overlap.
"""

import numpy as np

import concourse.bass as bass
import concourse.bacc as bacc
import concourse.mybir as mybir
import concourse.tile as tile
from concourse.masks import make_identity
from concourse.bass_utils import run_bass_kernel_spmd

B, S, H = 32, 1, 2048
E, I = 8, 5632
T = B * S          # 32 tokens
P = 128            # partitions
NCORES = 8
SWIGLU_SCALE = 1.702

KH = H // P        # 16 contraction chunks over H
KI = I // P        # 44 contraction chunks over I
NW = 512           # moving-dim tile width
NT = I // NW       # 11 gate/up column slabs
ND = H // NW       # 4 down output tiles
XW = T + E         # packed xT+router width (40)
SLAB = KH * NW     # 8192 cols per gate/up slab tile

F32 = mybir.dt.float32
F16 = mybir.dt.float16
AX = mybir.AxisListType.X
AF = mybir.ActivationFunctionType
OP = mybir.AluOpType


def _build_nc() -> bass.Bass:
    nc = bacc.Bacc()

    WDT = F16
    xrw_d = nc.declare_dram_parameter("xrw", [P, KH * XW], WDT, isOutput=False)
    wg_d = nc.declare_dram_parameter("wg", [H, I], WDT, isOutput=False)
    wu_d = nc.declare_dram_parameter("wu", [H, I], WDT, isOutput=False)
    wd_d = nc.declare_dram_parameter("wd", [I, H], WDT, isOutput=False)
    out_d = nc.declare_dram_parameter("out", [T, H], F16, isOutput=True)

    with tile.TileContext(nc) as tc:
        with tc.tile_pool(name="const", bufs=1) as const:
            id_sb = const.tile([T, T], F32, name="id_sb")
            make_identity(nc, id_sb)

            xrw_sb = const.tile([P, KH * XW], WDT, name="xrw_sb")
            nc.sync.dma_start(out=xrw_sb, in_=xrw_d[:, :])

            interT_sb = const.tile([P, KI * T], WDT, name="interT_sb")
            comb_sb = const.tile([T, 1], F32, name="comb_sb")
            out_sb = const.tile([T, H], F16, name="out_sb")

            def xT_k(k):  # [128, 32] stationary activation chunk
                return xrw_sb[:, k * XW : k * XW + T]

            def rw_k(k):  # [128, 8] router weight chunk
                return xrw_sb[:, k * XW + T : (k + 1) * XW]

            # weight DMAs keep ~1-4KB descriptors: real HW moves small
            # descriptors at full rate, big (16KB) ones ~12% slower
            wg_cols = wg_d.rearrange("(k p) i -> p k i", p=P)
            wu_cols = wu_d.rearrange("(k p) i -> p k i", p=P)
            wd_rows = wd_d.rearrange("(q p) h -> p q h", p=P)
            wgp = tc.alloc_tile_pool(name="wgp", bufs=3)
            wup = tc.alloc_tile_pool(name="wup", bufs=3)
            wdp = tc.alloc_tile_pool(name="wdp", bufs=3)

            # ---------------- router: softmax + top-2 mask ----------------
            with (
                tc.tile_pool(name="rps", bufs=1, space="PSUM") as rps,
                tc.tile_pool(name="rsb", bufs=1) as rsb,
            ):
                # absorb the ident DMA tick on PE before anything else
                dmy_ps = rps.tile([T, T], F32, name="dmy_ps", tag="dmy")
                nc.tensor.transpose(dmy_ps, id_sb, id_sb)

                logits = rps.tile([T, E], F32, name="logits", tag="logits")
                for k in range(KH):
                    nc.tensor.matmul(
                        logits,
                        xT_k(k),
                        rw_k(k),
                        start=(k == 0),
                        stop=(k == KH - 1),
                    )
                # PSUM is read only by DVE (keeps later PE writers 1-wait)
                lg = rsb.tile([T, E], F32, name="lg")
                nc.vector.tensor_copy(lg, logits)
                mx = rsb.tile([T, 1], F32, name="mx")
                nc.vector.reduce_max(mx, lg, axis=AX)
                nmx = rsb.tile([T, 1], F32, name="nmx")
                nc.vector.tensor_scalar_mul(nmx, mx, -1.0)
                ex = rsb.tile([T, E], F32, name="ex")
                nc.scalar.activation(ex, lg, AF.Exp, bias=nmx, scale=1.0)
                sm = rsb.tile([T, 1], F32, name="sm")
                nc.vector.reduce_sum(sm, ex, axis=AX)
                rc = rsb.tile([T, 1], F32, name="rc")
                nc.vector.reciprocal(rc, sm)
                aff = rsb.tile([T, E], F32, name="aff")
                nc.vector.tensor_scalar_mul(aff, ex, rc)
                # top-2: value >= (second largest)
                m1 = rsb.tile([T, 1], F32, name="m1")
                nc.vector.reduce_max(m1, aff, axis=AX)
                pen = rsb.tile([T, E], F32, name="pen")
                nc.vector.tensor_scalar(
                    pen, aff, m1, -1e30, op0=OP.is_equal, op1=OP.mult
                )
                b2 = rsb.tile([T, E], F32, name="b2")
                nc.vector.tensor_add(b2, aff, pen)
                m2 = rsb.tile([T, 1], F32, name="m2")
                nc.vector.reduce_max(m2, b2, axis=AX)
                ge = rsb.tile([T, E], F32, name="ge")
                nc.vector.tensor_scalar(ge, aff, m2, None, op0=OP.is_ge)
                msk = rsb.tile([T, E], F32, name="msk")
                nc.vector.tensor_mul(msk, aff, ge)
                # rolled router weight puts the own expert at column 0.
                # ACT-engine Copy also prewarms its table for the tail copies.
                nc.scalar.activation(comb_sb, msk[:, 0:1], AF.Copy, scale=1.0)

            # ---- fused gate/up + swiglu + transpose + interleaved down ----
            # Slab n computes interT chunks 4n..4n+3. Down matmuls run on
            # the freshest interT: slab 0 feeds chunks 2,3 (pair on gpsimd);
            # slabs 1..10 feed their own 4 chunks (2 pairs each, gpsimd);
            # chunks 0 (sync) and 1 (gpsimd) stream LAST so the post-stream
            # dependent chain is only their 8 matmuls + output copies.
            # Accumulation order per PSUM bank: 2,3,4..43,0,1 (start at 2,
            # stop at 1). PSUM: gate/up 2 + transpose 2 + down accum 4 = 8.
            with (
                tc.tile_pool(name="gup", bufs=1, space="PSUM") as gup,
                tc.tile_pool(name="tps", bufs=2, space="PSUM") as tps,
                tc.tile_pool(name="dps", bufs=1, space="PSUM") as dps,
                tc.tile_pool(name="esb", bufs=2) as esb,
            ):
                d_ps = [
                    dps.tile([T, NW], F32, name=f"d_ps{j}", tag=f"d{j}")
                    for j in range(ND)
                ]

                def down_mms(ki, wd_ap, c):
                    # 4 matmuls accumulating chunk ki from wd_ap's chunk c
                    for j in range(ND):
                        nc.tensor.matmul(
                            d_ps[j],
                            interT_sb[:, ki * T : (ki + 1) * T],
                            wd_ap[:, c * H + j * NW : c * H + (j + 1) * NW],
                            start=(ki == 2),
                            stop=(ki == 1),
                        )

                for n in range(NT):
                    wg_sl = wgp.tile([P, SLAB], WDT, name="wg_sl", tag="wg")
                    wu_sl = wup.tile([P, SLAB], WDT, name="wu_sl", tag="wu")
                    nc.sync.dma_start(
                        out=wg_sl.rearrange("p (k c) -> p k c", c=NW),
                        in_=wg_cols[:, :, n * NW : (n + 1) * NW],
                    )
                    nc.sync.dma_start(
                        out=wu_sl.rearrange("p (k c) -> p k c", c=NW),
                        in_=wu_cols[:, :, n * NW : (n + 1) * NW],
                    )
                    g_ps = gup.tile([T, NW], F32, name="g_ps", tag="g")
                    u_ps = gup.tile([T, NW], F32, name="u_ps", tag="u")
                    for k in range(KH):
                        nc.tensor.matmul(
                            g_ps,
                            xT_k(k),
                            wg_sl[:, k * NW : (k + 1) * NW],
                            start=(k == 0),
                            stop=(k == KH - 1),
                        )
                    for k in range(KH):
                        nc.tensor.matmul(
                            u_ps,
                            xT_k(k),
                            wu_sl[:, k * NW : (k + 1) * NW],
                            start=(k == 0),
                            stop=(k == KH - 1),
                        )
                    # epilogue: sigmoid runs off a copy; fold combine weight
                    g_sb = esb.tile([T, NW], F32, name="g_sb", tag="gsb")
                    nc.vector.tensor_copy(g_sb, g_ps)
                    sig = esb.tile([T, NW], F32, name="sig", tag="sig")
                    nc.scalar.activation(sig, g_sb, AF.Sigmoid, scale=SWIGLU_SCALE)
                    t1 = esb.tile([T, NW], F32, name="t1", tag="t1")
                    nc.vector.tensor_mul(t1, g_ps, sig)
                    t2 = esb.tile([T, NW], F32, name="t2", tag="t2")
                    nc.vector.tensor_mul(t2, t1, u_ps)
                    inter = esb.tile([T, NW], F32, name="inter", tag="inter")
                    nc.vector.tensor_scalar_mul(inter, t2, comb_sb)
                    for j in range(NW // P):
                        ic = 4 * n + j
                        tp = tps.tile([P, T], F32, name="tp", tag="tp")
                        nc.tensor.transpose(tp, inter[:, j * P : (j + 1) * P], id_sb)
                        nc.vector.tensor_copy(
                            interT_sb[:, ic * T : (ic + 1) * T], tp
                        )
                    # down-weight pairs + matmuls for this slab's chunks
                    if n == 0:
                        kis = [(2, 3)]      # chunks 0,1 deferred to the end
                    else:
                        kis = [(4 * n, 4 * n + 1), (4 * n + 2, 4 * n + 3)]
                    for k0, k1 in kis:
                        wd_pr = wdp.tile([P, 2 * H], WDT, name="wd_pr", tag="wdpr")
                        nc.gpsimd.dma_start(
                            out=wd_pr.rearrange("p (q h) -> p q h", h=H),
                            in_=wd_rows[:, k0 : k1 + 1, :],
                        )
                        down_mms(k0, wd_pr, 0)
                        down_mms(k1, wd_pr, 1)

                # final chunks 0 (sync queue) and 1 (gpsimd queue)
                wd_c0 = wdp.tile([P, H], WDT, name="wd_c0", tag="wds0")
                nc.sync.dma_start(out=wd_c0, in_=wd_d[0:P, :])
                wd_c1 = wdp.tile([P, H], WDT, name="wd_c1", tag="wds1")
                nc.gpsimd.dma_start(out=wd_c1, in_=wd_d[P : 2 * P, :])
                down_mms(0, wd_c0, 0)
                down_mms(1, wd_c1, 0)

                # output: PSUM -> fp16 SBUF (alternate DVE/ACT) -> DRAM
                for j in range(ND):
                    if j % 2 == 0:
                        nc.vector.tensor_copy(
                            out_sb[:, j * NW : (j + 1) * NW], d_ps[j]
                        )
                    else:
                        nc.scalar.activation(
                            out_sb[:, j * NW : (j + 1) * NW], d_ps[j],
                            AF.Copy, scale=1.0,
                        )
                    nc.sync.dma_start(
                        out=out_d[:, j * NW : (j + 1) * NW],
                        in_=out_sb[:, j * NW : (j + 1) * NW],
                    )
            wdp.release()
            wup.release()
            wgp.release()
    nc.finalize()
    return nc


def _pack_rows(a: np.ndarray) -> np.ndarray:
    """[K*P, C] row-major -> [P, K*C] SBUF tile layout (fp16)."""
    kp, c = a.shape
    k = kp // P
    return np.ascontiguousarray(
        a.reshape(k, P, c).transpose(1, 0, 2).reshape(P, k * c)
    )


def _make_in_maps(hidden_states, router_weight, w_gate, w_up, w_down):
    x = np.asarray(hidden_states, np.float32).reshape(T, H)
    rw = np.asarray(router_weight, np.float32)
    wg = np.asarray(w_gate, np.float16)
    wu = np.asarray(w_up, np.float16)
    wd = np.asarray(w_down, np.float16)
    xT = x.T.astype(np.float16)  # [H, T]

    in_maps = []
    for c in range(NCORES):
        order = [(j + c) % E for j in range(E)]  # column j holds expert (j+c)%E
        rwT = rw[order].T.astype(np.float16)  # [H, E]; col 0 = own expert
        xrw = _pack_rows(
            np.ascontiguousarray(np.concatenate([xT, rwT], axis=1))
        )  # [P, KH*XW]
        in_maps.append(
            {
                "xrw": xrw,
                "wg": np.ascontiguousarray(wg[c]),
                "wu": np.ascontiguousarray(wu[c]),
                "wd": np.ascontiguousarray(wd[c]),
            }
        )
    return in_maps


def kernel(
    hidden_states,
    router_weight,
    w_gate,
    w_up,
    w_down,
    top_k,
    _trace: bool = False,
    _trace_all: bool = False,
    **_unused,
):
    assert int(top_k) == 2, "kernel hardcodes top_k=2"
    in_maps = _make_in_maps(hidden_states, router_weight, w_gate, w_up, w_down)
    nc = _build_nc()
    res = run_bass_kernel_spmd(
        nc, in_maps, core_ids=list(range(NCORES)), trace=_trace,
        trace_cores=list(range(NCORES)) if (_trace and _trace_all) else None,
    )
    outs = np.stack([res.results[c]["out"] for c in range(NCORES)], axis=0)
    out = outs.astype(np.float64).sum(axis=0).astype(np.float32)
    if _trace:
        kernel.last_exec_time_ns = res.exec_time_ns
        kernel.last_mean_exec_time_ns = res.mean_exec_time_ns
        kernel.last_trace = res.instructions_and_trace
    return out.reshape(B, S, H)
